# revision 2
# baseline (speedup 1.0000x reference)
"""Binarized linear + BatchNorm via 1-level Strassen on 8 TRN2 cores.

out = BN(sign(x) @ sign(weight).T), x [8192,4096], weight [4096,4096].

Sharding: batch 4-way x out_features 2-way (as baseline). Per core:
C [2048(M) x 2048(N)] = A [2048x4096] @ B [4096x2048] with A=sign(W) shard,
B=sign(x).T shard.

Strassen (classic, 1 level, quarters M/2=1024, K/2=2048, N/2=1024):
  M1=(A11+A22)(B11+B22) M2=(A21+A22)B11 M3=A11(B12-B22) M4=A22(B21-B11)
  M5=(A11+A12)B22       M6=(A21-A11)(B11+B12)           M7=(A12-A22)(B21+B22)
  C11=M1+M4-M5+M7  C12=M3+M5  C21=M2+M4  C22=M1-M2+M3+M6
All operand values are in {-2,-1,0,1,2} (exact fp8e4); products accumulate
exactly in f32 PSUM (|M| <= 8192); C is an even integer |C| <= 4096 so the
f16 output is exact pre-BN. The 7 W-operands and 7 X-operands are formed on
the HOST and shipped as fp8.

PE work: 16 micro-steps x 7 products x 8 DR passes x 512 cols = 458752
column-cycles ~= 191us (vs 218us direct). Combines ride vector (tensor_tensor
PSUM+SBUF) + scalar (PSUM->SBUF copies), ~7us/step vs 12.1us PE/step.
"""

import numpy as np
from contextlib import ExitStack

B_FULL, IN, OUT = 8192, 4096, 4096
NB_CORES = 8
BI, OI = 4, 2
BS = B_FULL // BI        # 2048 batch per core (N)
OS = OUT // OI           # 2048 out per core (M)
MH, KH, NH = 1024, 2048, 1024   # Strassen half sizes
NOT = MH // 128          # 8 ot strips
NBT = NH // 512          # 2 bt strips
NST = KH // 256          # 8 DR supertile passes per product
BN_EPS = 1e-05

# product index computed at slot s (chosen so M3's combine finishes early
# and banks free before their next-step reuse)
SLOTS = [2, 3, 5, 4, 1, 6, 7]

_CACHE = {}


def _build_program():
    import concourse.tile as tile
    from concourse import mybir, bacc

    F8 = mybir.dt.float8e4
    F16 = mybir.dt.float16
    F32 = mybir.dt.float32
    ADD = mybir.AluOpType.add
    SUB = mybir.AluOpType.subtract
    MULT = mybir.AluOpType.mult
    DR = mybir.MatmulPerfMode.DoubleRow

    nc = bacc.Bacc("TRN2", target_bir_lowering=False, debug=False,
                   num_devices=NB_CORES)
    # X operands, slot-major: xq[s, p, bt, st2, n] (1MB per slot)
    xq = nc.declare_dram_parameter("xq", [7, 128, NBT, 16, 512], F8,
                                   isOutput=False)
    # W operands, ot-major chunks in slot order: wq[ot, p, s, st2, m]
    wq = nc.declare_dram_parameter("wq", [NOT, 128, 7, 16, 128], F8,
                                   isOutput=False)
    abv = nc.declare_dram_parameter("abv", [128, 2 * 16], F32, isOutput=False)
    o = nc.declare_dram_parameter("o", [OS, BS], F16, isOutput=True)
    dbg = nc.declare_dram_parameter("dbg", [128, 64], F16, isOutput=True)

    with tile.TileContext(nc) as tc:
        with ExitStack() as ctx:
            cons = ctx.enter_context(tc.tile_pool(name="cons", bufs=1))
            wchp = ctx.enter_context(tc.tile_pool(name="wch", bufs=3))
            scrp = ctx.enter_context(tc.tile_pool(name="scr", bufs=2))
            obp = ctx.enter_context(tc.tile_pool(name="ob", bufs=4))
            psp = ctx.enter_context(tc.tile_pool(name="ps", bufs=1,
                                                 space="PSUM"))

            ab_sb = cons.tile([128, 2 * 16], F32)
            xb = cons.tile([128, 7, NBT, 16, 512], F8)   # 112KB/partition
            dum = cons.tile([128, 2, 640], F8)

            nc.gpsimd.dma_start(ab_sb[:], abv[:])

            def x_dma(s, bt, ring):
                ring.dma_start(xb[:, s, bt], xq[s, :, bt])

            # --- W chunk tiles (one per (bt, ot) micro-step) ---
            wch = {}
            for bt in range(NBT):
                for ot in range(NOT):
                    wch[(bt, ot)] = wchp.tile([128, 7, 16, 128], F8,
                                              tag="wch",
                                              name=f"wch_{bt}_{ot}")

            # Input DMA issue. Phase 0 (below) interleaves micro-steps
            # (0,0)+(0,1) product-major, so fill-phase demand is a flat
            # ~290 GB/s (X strip + W0/W1 slot per 3.46us) against ~280-350
            # GB/s from the two HWDGE rings. Issue strictly in consumption
            # order, alternating rings item-wise; W2/W3 stay per-slot so
            # their slots land fluidly; chunks 4+ coarse alternating.
            _ri = [0]

            def alt():
                _ri[0] ^= 1
                return nc.sync if _ri[0] else nc.scalar

            for s in range(7):
                x_dma(s, 0, alt())
                alt().dma_start(wch[(0, 0)][:, s], wq[0, :, s])
                alt().dma_start(wch[(0, 1)][:, s], wq[1, :, s])
            for ck in (2, 3):
                for s in range(7):
                    alt().dma_start(wch[(0, ck)][:, s], wq[ck, :, s])
            for ck in range(4, NOT):
                ring = nc.sync if ck % 2 == 0 else nc.scalar
                ring.dma_start(wch[(0, ck)][:], wq[ck])

            # --- warm matmuls (keep PE busy / HAM warm until inputs land).
            # All warm MMs form ONE accumulation group into a dedicated bank
            # (open across phase 0), with a real DRAM reader at the end so
            # dead-code elimination cannot prune them.
            warm = psp.tile([128, 512], F32, tag="warm", bufs=1, name="warm")
            wsink = cons.tile([128, 64], F16)
            nc.vector.memset(dum[:], 1.0)
            wst = [False]

            def warm_mm(n):
                for _ in range(n):
                    nc.tensor.matmul(warm[:], dum[:, :, 0:128],
                                     dum[:, :, 128:640],
                                     start=(not wst[0]), stop=False,
                                     perf_mode=DR, skip_group_check=True)
                    wst[0] = True

            def warm_close():
                nc.tensor.matmul(warm[:], dum[:, :, 0:128],
                                 dum[:, :, 128:640],
                                 start=False, stop=True, perf_mode=DR,
                                 skip_group_check=True)
                nc.vector.tensor_copy(wsink[:], warm[:, 0:64])
                nc.gpsimd.dma_start(dbg[:], wsink[:])

            warm_mm(18)

            # --- phase 0: micro-steps (0,0)+(0,1) interleaved product-major
            # Each product runs for BOTH ot strips before the next X strip
            # is needed, halving the fill-phase X demand rate. M2/M3/M4/M5
            # drain to SBUF (scalar) right after their stop so PSUM stays
            # within 8 banks: long-lived M1/M6/M7 x2 strips take tags
            # m1..m6, short-lived products alternate m7/warm.
            PH_TAGS = {(1, 0): "m1", (1, 1): "m2", (6, 0): "m3",
                       (6, 1): "m4", (7, 0): "m5", (7, 1): "m6"}
            ph_sm = {}
            ph_pm = {}
            ph_t = {}
            nshort = 0
            for s in range(7):
                prod = SLOTS[s]
                for ot in range(2):
                    if prod in (2, 3, 4, 5):
                        tag = "m7"
                        nshort += 1
                    else:
                        tag = PH_TAGS[(prod, ot)]
                    acc = psp.tile([128, 512], F32, tag=tag, bufs=1,
                                   name=f"ph_m{prod}_{ot}")
                    for st in range(NST):
                        nc.tensor.matmul(
                            acc[:],
                            wch[(0, ot)][:, s, 2 * st:2 * st + 2, :],
                            xb[:, s, 0, 2 * st:2 * st + 2, :],
                            start=(st == 0), stop=(st == NST - 1),
                            perf_mode=DR)
                        if s == 0 or (s == 1 and st % 2 == 0):
                            warm_mm(1)
                    if prod in (2, 3, 4, 5):
                        smt = scrp.tile([128, 512], F32, tag=f"sm{prod}",
                                        name=f"ph_sm{prod}_{ot}")
                        nc.vector.tensor_copy(smt[:], acc[:])
                        ph_sm[(prod, ot)] = smt
                    else:
                        ph_pm[(prod, ot)] = acc
                    if prod == 5:
                        t12 = scrp.tile([128, 512], F32, tag="tA",
                                        name=f"ph_t12_{ot}")
                        nc.gpsimd.tensor_tensor(t12[:], ph_sm[(3, ot)][:],
                                                smt[:], ADD)
                        _bn_out(nc, tc, mybir, obp, ab_sb, t12, ot, 0,
                                o, 0, 1, eng=nc.gpsimd)
                    elif prod == 4:
                        t21 = scrp.tile([128, 512], F32, tag="tA",
                                        name=f"ph_t21_{ot}")
                        nc.gpsimd.tensor_tensor(t21[:], ph_sm[(2, ot)][:],
                                                smt[:], ADD)
                        _bn_out(nc, tc, mybir, obp, ab_sb, t21, ot, 0,
                                o, 1, 0, eng=nc.gpsimd)
                    elif prod == 1:
                        ta = scrp.tile([128, 512], F32, tag="tB",
                                       name=f"ph_ta_{ot}")
                        nc.vector.tensor_tensor(ta[:], acc[:],
                                                ph_sm[(5, ot)][:], SUB)
                        ph_t[("ta", ot)] = ta
                    elif prod == 6:
                        td = scrp.tile([128, 512], F32, tag="tC",
                                       name=f"ph_td_{ot}")
                        nc.vector.tensor_tensor(td[:], acc[:],
                                                ph_sm[(2, ot)][:], SUB)
                        te = scrp.tile([128, 512], F32, tag="tD",
                                       name=f"ph_te_{ot}")
                        nc.vector.tensor_tensor(te[:], ph_pm[(1, ot)][:],
                                                td[:], ADD)
                        tf = scrp.tile([128, 512], F32, tag="tC",
                                       name=f"ph_tf_{ot}")
                        nc.gpsimd.tensor_tensor(tf[:], ph_sm[(3, ot)][:],
                                                te[:], ADD)
                        _bn_out(nc, tc, mybir, obp, ab_sb, tf, ot, 0,
                                o, 1, 1, eng=nc.gpsimd)
                    elif prod == 7:
                        tb = scrp.tile([128, 512], F32, tag="tD",
                                       name=f"ph_tb_{ot}")
                        nc.vector.tensor_tensor(tb[:], acc[:],
                                                ph_t[("ta", ot)][:], ADD)
                        tc_ = scrp.tile([128, 512], F32, tag="tB",
                                        name=f"ph_tc_{ot}")
                        nc.vector.tensor_tensor(tc_[:], ph_sm[(4, ot)][:],
                                                tb[:], ADD)
                        _bn_out(nc, tc, mybir, obp, ab_sb, tc_, ot, 0,
                                o, 0, 0)

            warm_close()

            # --- main loop (steps (0,2)..(1,7)) ---
            for bt in range(NBT):
                for ot in range(NOT):
                    if bt == 0 and ot < 2:
                        continue
                    # deferred bt1 input issue. X odds ride the slow SWDGE
                    # ring from step 2 (plenty of lead time); X evens on sync
                    # at step 3. W bt1 chunk (1,ck) is issued only once its
                    # wch buffer's previous user ((0,ck+5) for bufs=3) is
                    # about to finish, so the descriptor never parks long on
                    # the ring queue and outputs behind it are not delayed.
                    if bt == 0 and ot == 2:
                        x_dma(1, 1, nc.scalar)
                        x_dma(3, 1, nc.scalar)
                        x_dma(5, 1, nc.scalar)
                    if bt == 0 and ot == 3:
                        for s in range(0, 7, 2):
                            x_dma(s, 1, nc.sync)
                    if bt == 0 and ot >= 6:
                        ck = ot - 6          # (1,0) at step 6, (1,1) at 7
                        ring = nc.sync if ck % 2 == 0 else nc.scalar
                        ring.dma_start(wch[(1, ck)][:], wq[ck])
                    if bt == 1 and ot <= 5:
                        ck = ot + 2          # (1,2) at step 8 ... (1,7) at 13
                        ring = nc.sync if ck % 2 == 0 else nc.scalar
                        ring.dma_start(wch[(1, ck)][:], wq[ck])
                    wt = wch[(bt, ot)]
                    lastst = (bt == NBT - 1 and ot == NOT - 1)
                    cmb = nc.vector if lastst else nc.gpsimd
                    ceng = None if lastst else nc.gpsimd
                    pm = {}
                    for s in range(7):
                        prod = SLOTS[s]
                        acc = psp.tile([128, 512], F32, tag=f"m{prod}",
                                       bufs=1, name=f"m{prod}_{bt}_{ot}")
                        pm[prod] = acc
                        for st in range(NST):
                            nc.tensor.matmul(
                                acc[:],
                                wt[:, s, 2 * st:2 * st + 2, :],
                                xb[:, s, bt, 2 * st:2 * st + 2, :],
                                start=(st == 0), stop=(st == NST - 1),
                                perf_mode=DR)
                            if bt == 0 and ot == 0:
                                # cover the fill-phase chase (~3-4us)
                                warm_mm(1 if s == 0 else
                                        (1 if s == 1 and st % 2 == 0 else 0))
                        # combines interleaved right after the producing slot
                        if prod == 2:
                            sm2 = scrp.tile([128, 512], F32, tag="sm2",
                                            name=f"sm2_{bt}_{ot}")
                            nc.vector.tensor_copy(sm2[:], pm[2][:])
                        elif prod == 3:
                            sm3 = scrp.tile([128, 512], F32, tag="sm3",
                                            name=f"sm3_{bt}_{ot}")
                            nc.vector.tensor_copy(sm3[:], pm[3][:])
                        elif prod == 5:
                            sm5 = scrp.tile([128, 512], F32, tag="sm5",
                                            name=f"sm5_{bt}_{ot}")
                            nc.vector.tensor_copy(sm5[:], pm[5][:])
                            t12 = scrp.tile([128, 512], F32, tag="tA",
                                            name=f"t12_{bt}_{ot}")
                            cmb.tensor_tensor(t12[:], sm3[:], sm5[:],
                                                    ADD)
                            _bn_out(nc, tc, mybir, obp, ab_sb, t12, ot, bt,
                                    o, 0, 1, eng=ceng)  # C12
                        elif prod == 4:
                            t21 = scrp.tile([128, 512], F32, tag="tA",
                                            name=f"t21_{bt}_{ot}")
                            nc.vector.tensor_tensor(t21[:], pm[4][:], sm2[:],
                                                    ADD)
                            _bn_out(nc, tc, mybir, obp, ab_sb, t21, ot, bt,
                                    o, 1, 0, eng=ceng)  # C21
                        elif prod == 1:
                            # pre-combine everything not needing M6/M7 so
                            # the post-M6/M7 vector chains are short (the
                            # last step's chain is the kernel tail)
                            ta = scrp.tile([128, 512], F32, tag="tB",
                                           name=f"ta_{bt}_{ot}")
                            nc.vector.tensor_tensor(ta[:], pm[1][:], sm5[:],
                                                    SUB)
                            v1 = scrp.tile([128, 512], F32, tag="tC",
                                           name=f"v1_{bt}_{ot}")
                            nc.vector.tensor_tensor(v1[:], pm[4][:], ta[:],
                                                    ADD)   # M1-M5+M4
                            u = scrp.tile([128, 512], F32, tag="tB",
                                          name=f"u_{bt}_{ot}")
                            nc.vector.tensor_tensor(u[:], pm[1][:], sm2[:],
                                                    SUB)   # M1-M2
                        elif prod == 6:
                            te = scrp.tile([128, 512], F32, tag="tD",
                                           name=f"te_{bt}_{ot}")
                            nc.vector.tensor_tensor(te[:], pm[6][:], u[:],
                                                    ADD)   # M1-M2+M6
                            tf = scrp.tile([128, 512], F32, tag="tB",
                                           name=f"tf_{bt}_{ot}")
                            cmb.tensor_tensor(tf[:], sm3[:], te[:],
                                                    ADD)
                            _bn_out(nc, tc, mybir, obp, ab_sb, tf, ot, bt,
                                    o, 1, 1, eng=ceng)  # C22
                        elif prod == 7:
                            tc_ = scrp.tile([128, 512], F32, tag="tD",
                                            name=f"tc_{bt}_{ot}")
                            nc.vector.tensor_tensor(tc_[:], pm[7][:], v1[:],
                                                    ADD)
                            _bn_out(nc, tc, mybir, obp, ab_sb, tc_, ot, bt,
                                    o, 0, 0)   # C11

    nc.compile()
    return nc


def _bn_out(nc, tc, mybir, obp, ab_sb, pre, ot, bt, o, rhalf, chalf,
            eng=None):
    """BN (a*x+b) -> f16 tile -> DMA to o[row block, col block]."""
    F16 = mybir.dt.float16
    r = rhalf * 8 + ot
    # ring split: C12/C21 (whose BNs run on gpsimd) ride the SWDGE ring so
    # any SWDGE backlog stalls only the gpsimd queue; C11 rides sync, C22
    # scalar. Separate ob tags per ring family contain backpressure. Last
    # micro-step's outputs all ride the fast HWDGE rings (tail drain).
    last = (bt == NBT - 1 and ot == NOT - 1)
    if last:
        ring, tag = (nc.sync, "obf") if (rhalf + chalf) % 2 == 0 else             (nc.scalar, "obf")
    elif rhalf == 0 and chalf == 1:
        ring, tag = nc.gpsimd, "obg"      # C12 (BN on gpsimd, ~10 GB/s)
    elif rhalf == 1 and chalf == 1:
        ring, tag = nc.scalar, "obf"      # C22
    else:
        ring, tag = nc.sync, "obf"        # C11 / C21
    ob = obp.tile([128, 512], F16, tag=tag, name=f"ob_{rhalf}{chalf}_{bt}_{ot}")
    (eng or nc.vector).tensor_scalar(
        ob[:], pre[:], ab_sb[:, r:r + 1], ab_sb[:, 16 + r:16 + r + 1],
        mybir.AluOpType.mult, mybir.AluOpType.add)
    ring.dma_start(
        o[rhalf * 1024 + ot * 128: rhalf * 1024 + (ot + 1) * 128,
          chalf * 1024 + bt * 512: chalf * 1024 + bt * 512 + 512],
        ob[:])


def make_in_maps(x, weight, bn_gamma, bn_beta, bn_mean, bn_var):
    import ml_dtypes
    f8 = getattr(ml_dtypes, "float8_e4m3", None) or ml_dtypes.float8_e4m3fn

    xs = np.sign(x).astype(np.int8)
    ws = np.sign(weight).astype(np.int8)
    std = np.sqrt(bn_var + np.float32(BN_EPS))
    a_full = bn_gamma / std
    b_full = bn_beta - bn_mean * a_full

    def x_image(Xi):
        # Xi [2048(k), 1024(n)] -> [128(p), 2(bt), 16(st2), 512]
        t = Xi.reshape(8, 2, 128, 2, 512).transpose(2, 3, 0, 1, 4)
        return np.ascontiguousarray(t.reshape(128, 2, 16, 512))

    def w_image(Wi_ot):
        # Wi_ot [128(m=q), 2048(k)] -> [128(p), 16(st2), 128(q)]
        t = Wi_ot.reshape(128, 8, 2, 128).transpose(3, 1, 2, 0)
        return np.ascontiguousarray(t.reshape(128, 16, 128))

    # X operands per batch shard
    xqs = []
    for bi in range(BI):
        Bm = xs[bi * BS:(bi + 1) * BS, :].T.astype(np.int16)  # [4096, 2048]
        B11 = Bm[:KH, :NH]; B12 = Bm[:KH, NH:]
        B21 = Bm[KH:, :NH]; B22 = Bm[KH:, NH:]
        ops = {1: B11 + B22, 2: B11, 3: B12 - B22, 4: B21 - B11,
               5: B22, 6: B11 + B12, 7: B21 + B22}
        arr = np.empty((7, 128, 2, 16, 512), dtype=f8)
        for s, prod in enumerate(SLOTS):
            arr[s] = x_image(ops[prod].astype(np.float32)).astype(f8)
        xqs.append(arr)

    # W operands + BN per out shard
    wqs, abs_ = [], []
    for oi in range(OI):
        Am = ws[oi * OS:(oi + 1) * OS, :].astype(np.int16)  # [2048, 4096]
        A11 = Am[:MH, :KH]; A12 = Am[:MH, KH:]
        A21 = Am[MH:, :KH]; A22 = Am[MH:, KH:]
        ops = {1: A11 + A22, 2: A21 + A22, 3: A11, 4: A22,
               5: A11 + A12, 6: A21 - A11, 7: A12 - A22}
        arr = np.empty((NOT, 128, 7, 16, 128), dtype=f8)
        for s, prod in enumerate(SLOTS):
            W = ops[prod].astype(np.float32)
            for ot in range(NOT):
                arr[ot, :, s] = w_image(W[ot * 128:(ot + 1) * 128, :]).astype(f8)
        wqs.append(arr)
        sl = slice(oi * OS, (oi + 1) * OS)
        abs_.append(np.ascontiguousarray(np.concatenate(
            [a_full[sl].reshape(16, 128).T, b_full[sl].reshape(16, 128).T],
            axis=1, dtype=np.float32)))

    in_maps = []
    for c in range(NB_CORES):
        bi, oi = divmod(c, OI)
        in_maps.append({"xq": xqs[bi], "wq": wqs[oi], "abv": abs_[oi]})
    return in_maps


def kernel(x, weight, bn_gamma, bn_beta, bn_mean, bn_var):
    from concourse.bass_utils import run_bass_kernel_spmd

    x = np.asarray(x, dtype=np.float32)
    weight = np.asarray(weight, dtype=np.float32)
    bn_gamma = np.asarray(bn_gamma, dtype=np.float32)
    bn_beta = np.asarray(bn_beta, dtype=np.float32)
    bn_mean = np.asarray(bn_mean, dtype=np.float32)
    bn_var = np.asarray(bn_var, dtype=np.float32)

    if "nc" not in _CACHE:
        _CACHE["nc"] = _build_program()
    nc = _CACHE["nc"]

    in_maps = make_in_maps(x, weight, bn_gamma, bn_beta, bn_mean, bn_var)

    res = run_bass_kernel_spmd(nc, in_maps, list(range(NB_CORES)))
    _CACHE["last_results"] = res

    out = np.empty((B_FULL, OUT), dtype=np.float32)
    for c in range(NB_CORES):
        bi, oi = divmod(c, OI)
        out[bi * BS:(bi + 1) * BS, oi * OS:(oi + 1) * OS] = \
            res.results[c]["o"].T.astype(np.float32)
    return out


# revision 3
# speedup vs baseline: 1.0103x; 1.0103x over previous
"""Binarized linear + BatchNorm via 1-level Strassen on 8 TRN2 cores.

out = BN(sign(x) @ sign(weight).T), x [8192,4096], weight [4096,4096].

Sharding: batch 4-way x out_features 2-way (as baseline). Per core:
C [2048(M) x 2048(N)] = A [2048x4096] @ B [4096x2048] with A=sign(W) shard,
B=sign(x).T shard.

Strassen (classic, 1 level, quarters M/2=1024, K/2=2048, N/2=1024):
  M1=(A11+A22)(B11+B22) M2=(A21+A22)B11 M3=A11(B12-B22) M4=A22(B21-B11)
  M5=(A11+A12)B22       M6=(A21-A11)(B11+B12)           M7=(A12-A22)(B21+B22)
  C11=M1+M4-M5+M7  C12=M3+M5  C21=M2+M4  C22=M1-M2+M3+M6
All operand values are in {-2,-1,0,1,2} (exact fp8e4); products accumulate
exactly in f32 PSUM (|M| <= 8192); C is an even integer |C| <= 4096 so the
f16 output is exact pre-BN. The 7 W-operands and 7 X-operands are formed on
the HOST and shipped as fp8.

PE work: 16 micro-steps x 7 products x 8 DR passes x 512 cols = 458752
column-cycles ~= 191us (vs 218us direct). PSUM drains + PSUM-side combines
ride the vector engine; SBUF-side combines and 3 of 4 BNs ride gpsimd; the
sync/scalar engines are pure DMA rings (a non-DMA op queued behind their
sem-recycled descriptors would stall the PE via buffer backpressure).
Micro-steps (0,0)+(0,1) run interleaved product-major so fill-phase X
demand is flat; one warm-matmul accumulation group (with a real DRAM
reader, else it is dead-code-eliminated) covers the input-gated windows.
"""

import numpy as np
from contextlib import ExitStack

B_FULL, IN, OUT = 8192, 4096, 4096
NB_CORES = 8
BI, OI = 4, 2
BS = B_FULL // BI        # 2048 batch per core (N)
OS = OUT // OI           # 2048 out per core (M)
MH, KH, NH = 1024, 2048, 1024   # Strassen half sizes
NOT = MH // 128          # 8 ot strips
NBT = NH // 512          # 2 bt strips
NST = KH // 256          # 8 DR supertile passes per product
BN_EPS = 1e-05

# product index computed at slot s (chosen so M3's combine finishes early
# and banks free before their next-step reuse)
SLOTS = [2, 3, 5, 4, 1, 6, 7]

_CACHE = {}


def _build_program():
    import concourse.tile as tile
    from concourse import mybir, bacc

    F8 = mybir.dt.float8e4
    F16 = mybir.dt.float16
    F32 = mybir.dt.float32
    ADD = mybir.AluOpType.add
    SUB = mybir.AluOpType.subtract
    MULT = mybir.AluOpType.mult
    DR = mybir.MatmulPerfMode.DoubleRow

    nc = bacc.Bacc("TRN2", target_bir_lowering=False, debug=False,
                   num_devices=NB_CORES)
    # X operands, slot-major: xq[s, p, bt, st2, n] (1MB per slot)
    xq = nc.declare_dram_parameter("xq", [7, 128, NBT, 16, 512], F8,
                                   isOutput=False)
    # W operands, ot-major chunks in slot order: wq[ot, p, s, st2, m]
    wq = nc.declare_dram_parameter("wq", [NOT, 128, 7, 16, 128], F8,
                                   isOutput=False)
    abv = nc.declare_dram_parameter("abv", [128, 2 * 16], F32, isOutput=False)
    o = nc.declare_dram_parameter("o", [OS, BS], F16, isOutput=True)
    dbg = nc.declare_dram_parameter("dbg", [128, 64], F16, isOutput=True)

    with tile.TileContext(nc) as tc:
        with ExitStack() as ctx:
            cons = ctx.enter_context(tc.tile_pool(name="cons", bufs=1))
            wchp = ctx.enter_context(tc.tile_pool(name="wch", bufs=3))
            scrp = ctx.enter_context(tc.tile_pool(name="scr", bufs=2))
            obp = ctx.enter_context(tc.tile_pool(name="ob", bufs=4))
            psp = ctx.enter_context(tc.tile_pool(name="ps", bufs=1,
                                                 space="PSUM"))

            ab_sb = cons.tile([128, 2 * 16], F32)
            xb = cons.tile([128, 7, NBT, 16, 512], F8)   # 112KB/partition
            dum = cons.tile([128, 2, 640], F8)

            nc.gpsimd.dma_start(ab_sb[:], abv[:])

            def x_dma(s, bt, ring):
                ring.dma_start(xb[:, s, bt], xq[s, :, bt])

            # --- W chunk tiles (one per (bt, ot) micro-step) ---
            wch = {}
            for bt in range(NBT):
                for ot in range(NOT):
                    wch[(bt, ot)] = wchp.tile([128, 7, 16, 128], F8,
                                              tag="wch",
                                              name=f"wch_{bt}_{ot}")

            # Input DMA issue. Phase 0 (below) interleaves micro-steps
            # (0,0)+(0,1) product-major, so fill-phase demand is a flat
            # ~290 GB/s (X strip + W0/W1 slot per 3.46us) against ~280-350
            # GB/s from the two HWDGE rings. Issue strictly in consumption
            # order, alternating rings item-wise; W2/W3 stay per-slot so
            # their slots land fluidly; chunks 4+ coarse alternating.
            _ri = [0]

            def alt():
                _ri[0] ^= 1
                return nc.sync if _ri[0] else nc.scalar

            for s in range(7):
                x_dma(s, 0, alt())
                alt().dma_start(wch[(0, 0)][:, s], wq[0, :, s])
                alt().dma_start(wch[(0, 1)][:, s], wq[1, :, s])
            for ck in (2, 3):
                for s in range(7):
                    alt().dma_start(wch[(0, ck)][:, s], wq[ck, :, s])
            for ck in range(4, NOT):
                ring = nc.sync if ck % 2 == 0 else nc.scalar
                ring.dma_start(wch[(0, ck)][:], wq[ck])

            # --- warm matmuls (keep PE busy / HAM warm until inputs land).
            # All warm MMs form ONE accumulation group into a dedicated bank
            # (open across phase 0), with a real DRAM reader at the end so
            # dead-code elimination cannot prune them.
            warm = psp.tile([128, 512], F32, tag="warm", bufs=1, name="warm")
            wsink = cons.tile([128, 64], F16)
            nc.vector.memset(dum[:], 1.0)
            wst = [False]

            def warm_mm(n):
                for _ in range(n):
                    nc.tensor.matmul(warm[:], dum[:, :, 0:128],
                                     dum[:, :, 128:640],
                                     start=(not wst[0]), stop=False,
                                     perf_mode=DR, skip_group_check=True)
                    wst[0] = True

            def warm_close():
                nc.tensor.matmul(warm[:], dum[:, :, 0:128],
                                 dum[:, :, 128:640],
                                 start=False, stop=True, perf_mode=DR,
                                 skip_group_check=True)
                nc.vector.tensor_copy(wsink[:], warm[:, 0:64])
                nc.gpsimd.dma_start(dbg[:], wsink[:])

            warm_mm(18)

            # --- phase 0: micro-steps (0,0)+(0,1) interleaved product-major
            # Each product runs for BOTH ot strips before the next X strip
            # is needed, halving the fill-phase X demand rate. M2/M3/M4/M5
            # drain to SBUF (scalar) right after their stop so PSUM stays
            # within 8 banks: long-lived M1/M6/M7 x2 strips take tags
            # m1..m6, short-lived products alternate m7/warm.
            PH_TAGS = {(1, 0): "m1", (1, 1): "m2", (6, 0): "m3",
                       (6, 1): "m4", (7, 0): "m5", (7, 1): "m6"}
            ph_sm = {}
            ph_pm = {}
            ph_t = {}
            nshort = 0
            for s in range(7):
                prod = SLOTS[s]
                for ot in range(2):
                    if prod in (2, 3, 4, 5):
                        tag = "m7"
                        nshort += 1
                    else:
                        tag = PH_TAGS[(prod, ot)]
                    acc = psp.tile([128, 512], F32, tag=tag, bufs=1,
                                   name=f"ph_m{prod}_{ot}")
                    for st in range(NST):
                        nc.tensor.matmul(
                            acc[:],
                            wch[(0, ot)][:, s, 2 * st:2 * st + 2, :],
                            xb[:, s, 0, 2 * st:2 * st + 2, :],
                            start=(st == 0), stop=(st == NST - 1),
                            perf_mode=DR)
                        if s == 0 or (s == 1 and st % 2 == 0):
                            warm_mm(1)
                    if prod in (2, 3, 4, 5):
                        smt = scrp.tile([128, 512], F32, tag=f"sm{prod}",
                                        name=f"ph_sm{prod}_{ot}")
                        nc.vector.tensor_copy(smt[:], acc[:])
                        ph_sm[(prod, ot)] = smt
                    else:
                        ph_pm[(prod, ot)] = acc
                    if prod == 5:
                        t12 = scrp.tile([128, 512], F32, tag="tA",
                                        name=f"ph_t12_{ot}")
                        nc.gpsimd.tensor_tensor(t12[:], ph_sm[(3, ot)][:],
                                                smt[:], ADD)
                        _bn_out(nc, tc, mybir, obp, ab_sb, t12, ot, 0,
                                o, 0, 1, eng=nc.gpsimd)
                    elif prod == 4:
                        t21 = scrp.tile([128, 512], F32, tag="tA",
                                        name=f"ph_t21_{ot}")
                        nc.gpsimd.tensor_tensor(t21[:], ph_sm[(2, ot)][:],
                                                smt[:], ADD)
                        _bn_out(nc, tc, mybir, obp, ab_sb, t21, ot, 0,
                                o, 1, 0, eng=nc.gpsimd)
                    elif prod == 1:
                        ta = scrp.tile([128, 512], F32, tag="tB",
                                       name=f"ph_ta_{ot}")
                        nc.vector.tensor_tensor(ta[:], acc[:],
                                                ph_sm[(5, ot)][:], SUB)
                        ph_t[("ta", ot)] = ta
                    elif prod == 6:
                        td = scrp.tile([128, 512], F32, tag="tC",
                                       name=f"ph_td_{ot}")
                        nc.vector.tensor_tensor(td[:], acc[:],
                                                ph_sm[(2, ot)][:], SUB)
                        te = scrp.tile([128, 512], F32, tag="tD",
                                       name=f"ph_te_{ot}")
                        nc.vector.tensor_tensor(te[:], ph_pm[(1, ot)][:],
                                                td[:], ADD)
                        tf = scrp.tile([128, 512], F32, tag="tC",
                                       name=f"ph_tf_{ot}")
                        nc.gpsimd.tensor_tensor(tf[:], ph_sm[(3, ot)][:],
                                                te[:], ADD)
                        _bn_out(nc, tc, mybir, obp, ab_sb, tf, ot, 0,
                                o, 1, 1, eng=nc.gpsimd)
                    elif prod == 7:
                        tb = scrp.tile([128, 512], F32, tag="tD",
                                       name=f"ph_tb_{ot}")
                        nc.vector.tensor_tensor(tb[:], acc[:],
                                                ph_t[("ta", ot)][:], ADD)
                        tc_ = scrp.tile([128, 512], F32, tag="tB",
                                        name=f"ph_tc_{ot}")
                        nc.vector.tensor_tensor(tc_[:], ph_sm[(4, ot)][:],
                                                tb[:], ADD)
                        _bn_out(nc, tc, mybir, obp, ab_sb, tc_, ot, 0,
                                o, 0, 0)

            warm_close()

            # --- main loop (steps (0,2)..(1,7)) ---
            for bt in range(NBT):
                for ot in range(NOT):
                    if bt == 0 and ot < 2:
                        continue
                    # deferred bt1 input issue. X odds ride the slow SWDGE
                    # ring from step 2 (plenty of lead time); X evens on sync
                    # at step 3. W bt1 chunk (1,ck) is issued only once its
                    # wch buffer's previous user ((0,ck+5) for bufs=3) is
                    # about to finish, so the descriptor never parks long on
                    # the ring queue and outputs behind it are not delayed.
                    if bt == 0 and ot == 2:
                        x_dma(1, 1, nc.scalar)
                        x_dma(3, 1, nc.scalar)
                        x_dma(5, 1, nc.scalar)
                    if bt == 0 and ot == 3:
                        for s in range(0, 7, 2):
                            x_dma(s, 1, nc.sync)
                    if bt == 0 and ot >= 6:
                        ck = ot - 6          # (1,0) at step 6, (1,1) at 7
                        ring = nc.sync if ck % 2 == 0 else nc.scalar
                        ring.dma_start(wch[(1, ck)][:], wq[ck])
                    if bt == 1 and ot <= 5:
                        ck = ot + 2          # (1,2) at step 8 ... (1,7) at 13
                        ring = nc.sync if ck % 2 == 0 else nc.scalar
                        ring.dma_start(wch[(1, ck)][:], wq[ck])
                    wt = wch[(bt, ot)]
                    lastst = (bt == NBT - 1 and ot == NOT - 1)
                    cmb = nc.vector if lastst else nc.gpsimd
                    ceng = None if lastst else nc.gpsimd
                    pm = {}
                    for s in range(7):
                        prod = SLOTS[s]
                        acc = psp.tile([128, 512], F32, tag=f"m{prod}",
                                       bufs=1, name=f"m{prod}_{bt}_{ot}")
                        pm[prod] = acc
                        for st in range(NST):
                            nc.tensor.matmul(
                                acc[:],
                                wt[:, s, 2 * st:2 * st + 2, :],
                                xb[:, s, bt, 2 * st:2 * st + 2, :],
                                start=(st == 0), stop=(st == NST - 1),
                                perf_mode=DR)
                            if bt == 0 and ot == 0:
                                # cover the fill-phase chase (~3-4us)
                                warm_mm(1 if s == 0 else
                                        (1 if s == 1 and st % 2 == 0 else 0))
                        # combines interleaved right after the producing slot
                        if prod == 2:
                            sm2 = scrp.tile([128, 512], F32, tag="sm2",
                                            name=f"sm2_{bt}_{ot}")
                            nc.vector.tensor_copy(sm2[:], pm[2][:])
                        elif prod == 3:
                            sm3 = scrp.tile([128, 512], F32, tag="sm3",
                                            name=f"sm3_{bt}_{ot}")
                            nc.vector.tensor_copy(sm3[:], pm[3][:])
                        elif prod == 5:
                            sm5 = scrp.tile([128, 512], F32, tag="sm5",
                                            name=f"sm5_{bt}_{ot}")
                            nc.vector.tensor_copy(sm5[:], pm[5][:])
                            t12 = scrp.tile([128, 512], F32, tag="tA",
                                            name=f"t12_{bt}_{ot}")
                            cmb.tensor_tensor(t12[:], sm3[:], sm5[:],
                                                    ADD)
                            _bn_out(nc, tc, mybir, obp, ab_sb, t12, ot, bt,
                                    o, 0, 1, eng=ceng)  # C12
                        elif prod == 4:
                            t21 = scrp.tile([128, 512], F32, tag="tA",
                                            name=f"t21_{bt}_{ot}")
                            nc.vector.tensor_tensor(t21[:], pm[4][:], sm2[:],
                                                    ADD)
                            _bn_out(nc, tc, mybir, obp, ab_sb, t21, ot, bt,
                                    o, 1, 0, eng=ceng)  # C21
                        elif prod == 1:
                            # pre-combine everything not needing M6/M7 so
                            # the post-M6/M7 vector chains are short (the
                            # last step's chain is the kernel tail)
                            ta = scrp.tile([128, 512], F32, tag="tB",
                                           name=f"ta_{bt}_{ot}")
                            nc.vector.tensor_tensor(ta[:], pm[1][:], sm5[:],
                                                    SUB)
                            v1 = scrp.tile([128, 512], F32, tag="tC",
                                           name=f"v1_{bt}_{ot}")
                            nc.vector.tensor_tensor(v1[:], pm[4][:], ta[:],
                                                    ADD)   # M1-M5+M4
                            u = scrp.tile([128, 512], F32, tag="tB",
                                          name=f"u_{bt}_{ot}")
                            nc.vector.tensor_tensor(u[:], pm[1][:], sm2[:],
                                                    SUB)   # M1-M2
                        elif prod == 6:
                            te = scrp.tile([128, 512], F32, tag="tD",
                                           name=f"te_{bt}_{ot}")
                            nc.vector.tensor_tensor(te[:], pm[6][:], u[:],
                                                    ADD)   # M1-M2+M6
                            tf = scrp.tile([128, 512], F32, tag="tB",
                                           name=f"tf_{bt}_{ot}")
                            cmb.tensor_tensor(tf[:], sm3[:], te[:],
                                                    ADD)
                            _bn_out(nc, tc, mybir, obp, ab_sb, tf, ot, bt,
                                    o, 1, 1, eng=ceng)  # C22
                        elif prod == 7:
                            tc_ = scrp.tile([128, 512], F32, tag="tD",
                                            name=f"tc_{bt}_{ot}")
                            nc.vector.tensor_tensor(tc_[:], pm[7][:], v1[:],
                                                    ADD)
                            _bn_out(nc, tc, mybir, obp, ab_sb, tc_, ot, bt,
                                    o, 0, 0)   # C11

    nc.compile()
    return nc


def _bn_out(nc, tc, mybir, obp, ab_sb, pre, ot, bt, o, rhalf, chalf,
            eng=None):
    """BN (a*x+b) -> f16 tile -> DMA to o[row block, col block]."""
    F16 = mybir.dt.float16
    r = rhalf * 8 + ot
    # ring split: C12/C21 (whose BNs run on gpsimd) ride the SWDGE ring so
    # any SWDGE backlog stalls only the gpsimd queue; C11 rides sync, C22
    # scalar. Separate ob tags per ring family contain backpressure. Last
    # micro-step's outputs all ride the fast HWDGE rings (tail drain).
    last = (bt == NBT - 1 and ot == NOT - 1)
    if last:
        ring, tag = (nc.sync, "obf") if (rhalf + chalf) % 2 == 0 else             (nc.scalar, "obf")
    elif rhalf == 0 and chalf == 1:
        ring, tag = nc.gpsimd, "obg"      # C12 (BN on gpsimd, ~10 GB/s)
    elif rhalf == 1 and chalf == 1:
        ring, tag = nc.scalar, "obf"      # C22
    else:
        ring, tag = nc.sync, "obf"        # C11 / C21
    ob = obp.tile([128, 512], F16, tag=tag, name=f"ob_{rhalf}{chalf}_{bt}_{ot}")
    (eng or nc.vector).tensor_scalar(
        ob[:], pre[:], ab_sb[:, r:r + 1], ab_sb[:, 16 + r:16 + r + 1],
        mybir.AluOpType.mult, mybir.AluOpType.add)
    ring.dma_start(
        o[rhalf * 1024 + ot * 128: rhalf * 1024 + (ot + 1) * 128,
          chalf * 1024 + bt * 512: chalf * 1024 + bt * 512 + 512],
        ob[:])


def make_in_maps(x, weight, bn_gamma, bn_beta, bn_mean, bn_var):
    import ml_dtypes
    f8 = getattr(ml_dtypes, "float8_e4m3", None) or ml_dtypes.float8_e4m3fn

    xs = np.sign(x).astype(np.int8)
    ws = np.sign(weight).astype(np.int8)
    std = np.sqrt(bn_var + np.float32(BN_EPS))
    a_full = bn_gamma / std
    b_full = bn_beta - bn_mean * a_full

    def x_image(Xi):
        # Xi [2048(k), 1024(n)] -> [128(p), 2(bt), 16(st2), 512]
        t = Xi.reshape(8, 2, 128, 2, 512).transpose(2, 3, 0, 1, 4)
        return np.ascontiguousarray(t.reshape(128, 2, 16, 512))

    def w_image(Wi_ot):
        # Wi_ot [128(m=q), 2048(k)] -> [128(p), 16(st2), 128(q)]
        t = Wi_ot.reshape(128, 8, 2, 128).transpose(3, 1, 2, 0)
        return np.ascontiguousarray(t.reshape(128, 16, 128))

    # X operands per batch shard
    xqs = []
    for bi in range(BI):
        Bm = xs[bi * BS:(bi + 1) * BS, :].T.astype(np.int16)  # [4096, 2048]
        B11 = Bm[:KH, :NH]; B12 = Bm[:KH, NH:]
        B21 = Bm[KH:, :NH]; B22 = Bm[KH:, NH:]
        ops = {1: B11 + B22, 2: B11, 3: B12 - B22, 4: B21 - B11,
               5: B22, 6: B11 + B12, 7: B21 + B22}
        arr = np.empty((7, 128, 2, 16, 512), dtype=f8)
        for s, prod in enumerate(SLOTS):
            arr[s] = x_image(ops[prod].astype(np.float32)).astype(f8)
        xqs.append(arr)

    # W operands + BN per out shard
    wqs, abs_ = [], []
    for oi in range(OI):
        Am = ws[oi * OS:(oi + 1) * OS, :].astype(np.int16)  # [2048, 4096]
        A11 = Am[:MH, :KH]; A12 = Am[:MH, KH:]
        A21 = Am[MH:, :KH]; A22 = Am[MH:, KH:]
        ops = {1: A11 + A22, 2: A21 + A22, 3: A11, 4: A22,
               5: A11 + A12, 6: A21 - A11, 7: A12 - A22}
        arr = np.empty((NOT, 128, 7, 16, 128), dtype=f8)
        for s, prod in enumerate(SLOTS):
            W = ops[prod].astype(np.float32)
            for ot in range(NOT):
                arr[ot, :, s] = w_image(W[ot * 128:(ot + 1) * 128, :]).astype(f8)
        wqs.append(arr)
        sl = slice(oi * OS, (oi + 1) * OS)
        abs_.append(np.ascontiguousarray(np.concatenate(
            [a_full[sl].reshape(16, 128).T, b_full[sl].reshape(16, 128).T],
            axis=1, dtype=np.float32)))

    in_maps = []
    for c in range(NB_CORES):
        bi, oi = divmod(c, OI)
        in_maps.append({"xq": xqs[bi], "wq": wqs[oi], "abv": abs_[oi]})
    return in_maps


def kernel(x, weight, bn_gamma, bn_beta, bn_mean, bn_var):
    from concourse.bass_utils import run_bass_kernel_spmd

    x = np.asarray(x, dtype=np.float32)
    weight = np.asarray(weight, dtype=np.float32)
    bn_gamma = np.asarray(bn_gamma, dtype=np.float32)
    bn_beta = np.asarray(bn_beta, dtype=np.float32)
    bn_mean = np.asarray(bn_mean, dtype=np.float32)
    bn_var = np.asarray(bn_var, dtype=np.float32)

    if "nc" not in _CACHE:
        _CACHE["nc"] = _build_program()
    nc = _CACHE["nc"]

    in_maps = make_in_maps(x, weight, bn_gamma, bn_beta, bn_mean, bn_var)

    res = run_bass_kernel_spmd(nc, in_maps, list(range(NB_CORES)))
    _CACHE["last_results"] = res

    out = np.empty((B_FULL, OUT), dtype=np.float32)
    for c in range(NB_CORES):
        bi, oi = divmod(c, OI)
        out[bi * BS:(bi + 1) * BS, oi * OS:(oi + 1) * OS] = \
            res.results[c]["o"].T.astype(np.float32)
    return out
